# revision 2
# baseline (speedup 1.0000x reference)
"""Trainium2 Bass kernel (v5) for MergedQKVParallelLinearWithLoRA.

v4 pipeline plus PSUM-bank-stride fix: consecutive accumulating matmuls
into the same PSUM bank stall the PE (~40ns/mm measured); rotating >=6
banks between same-bank touches runs at the streaming rate.

  - Phase 1: ONE hh pass, 6 interleaved accumulation rounds
    (j x tch), bank stride 6, lhsT paired over tch.  A-window tiles are
    streamed per hh (read once).  xT/A dma_starts interleave with the hh
    loop on SP so compute starts ~3us into the iteration.
  - Phase 2: 12 obpairs (2 weight blocks each) x 8 interleaved rounds
    (2 ob x 2 sb x 2 tch), bank stride 8, lhsT paired over tch.
  - Constants (bias/mask/bT) + weight blocks + out-stores on the ACT
    HWDGE queue; SP carries only the latency-critical xT/A stream.
"""

import numpy as np

T = 8192
H = 4096
OUT_Q = 4096
OUT_KV = 1024
OUT = OUT_Q + 2 * OUT_KV  # 6144
L = 16
R = 16
NCORES = 8
TC = T // NCORES  # 1024

NH = H // 128          # 32 contraction tiles
NOB = OUT // 256       # 24 weight blocks (256 cols each)
NOBP = NOB // 2        # 12 obpairs
NG = OUT // 128        # 48 output 128-row groups
NTCH = TC // 512       # 2 token chunks
NGQ = OUT_Q // 128     # 32
NGK = OUT_KV // 128    # 8

_cache = {}


def _build(reps=1, timing_inputs=False, skip_lora=False, skip_main=False, lw=8):
    import concourse.bass as bass  # noqa: F401
    import concourse.mybir as mybir
    import concourse.tile as tile
    from concourse import bacc

    f32 = mybir.dt.float32
    bf16 = mybir.dt.bfloat16

    lrw = lw * R               # lr rows per slice in window (128 or 256)
    nbt = lrw // 128           # expand matmuls per round (1 or 2)
    njw = 3 * nbt              # shrink j tiles (3 or 6)

    nc = bacc.Bacc(None, target_bir_lowering=False)

    in_kw = {} if timing_inputs else {"kind": "ExternalInput"}
    xTp = nc.dram_tensor("xTp", [128, NH, TC], bf16, **in_kw)
    wp = nc.dram_tensor("wp", [NOB, 128, NH, 256], bf16, **in_kw)
    aTp = nc.dram_tensor("aTp", [128, NH, njw * 128], bf16, **in_kw)
    bTp = nc.dram_tensor("bTp", [128, nbt, OUT], bf16, **in_kw)
    maskTp = nc.dram_tensor("maskTp", [128, njw, TC], bf16, **in_kw)
    biasP = nc.dram_tensor("biasP", [128, NG], f32, **in_kw)
    if timing_inputs:
        out = nc.dram_tensor("out", [OUT, TC], f32)
        sink = nc.dram_tensor("sink", [128, 512], f32, kind="ExternalOutput")
    else:
        out = nc.dram_tensor("out", [OUT, TC], f32, kind="ExternalOutput")
        sink = None

    with tile.TileContext(nc) as tc:
        from contextlib import ExitStack

        with ExitStack() as ctx:
            xp = ctx.enter_context(tc.tile_pool(name="xp", bufs=1))
            cp = ctx.enter_context(tc.tile_pool(name="cp", bufs=1))
            sp = ctx.enter_context(tc.tile_pool(name="sp", bufs=2))
            pp = ctx.enter_context(tc.tile_pool(name="pp", bufs=8, space="PSUM"))
            atp = ctx.enter_context(tc.tile_pool(name="atp", bufs=3))
            wpool = ctx.enter_context(tc.tile_pool(name="wpool", bufs=4))
            op = ctx.enter_context(tc.tile_pool(name="op", bufs=4))

            loop_ctx = tc.For_i(0, reps, 1) if reps > 1 else None
            if loop_ctx is not None:
                loop_ctx.__enter__()

            xT_sb = xp.tile([128, NH, TC], bf16, name="xT_sb", tag="xT_sb")
            bT_sb = cp.tile([128, nbt, OUT], bf16, name="bT_sb", tag="bT_sb")
            mask_sb = cp.tile([128, njw, TC], bf16, name="mask_sb", tag="mask_sb")
            bias_sb = cp.tile([128, NG], f32, name="bias_sb", tag="bias_sb")
            shrT = sp.tile([128, njw, TC], bf16, name="shrT", tag="shrT")

            def load_x(q):  # 4-hh slab of xT on SP
                nc.sync.dma_start(
                    xT_sb[:, q * 4:(q + 1) * 4, :], xTp[:, q * 4:(q + 1) * 4, :]
                )

            # constants on ACT, ahead of the weight stream
            nc.scalar.dma_start(bias_sb, biasP[:, :])
            nc.scalar.dma_start(mask_sb, maskTp[:, :, :])
            nc.scalar.dma_start(bT_sb, bTp[:, :, :])
            load_x(0)
            if skip_lora:
                for q in range(1, 8):
                    load_x(q)

            # ---- Phase 1: windowed LoRA shrink + mask (one hh pass) ----
            if not skip_lora:
                ps1 = [
                    pp.tile([128, 512], f32, name=f"shps_{r}", tag="ps")
                    for r in range(2 * njw)
                ]
                for hh in range(NH):
                    if hh % 4 == 0 and hh // 4 < 7:
                        load_x(hh // 4 + 1)
                    at = atp.tile(
                        [128, njw * 128], bf16, name=f"at_{hh}", tag="at"
                    )
                    nc.sync.dma_start(at, aTp[:, hh, :])
                    for j in range(njw):
                        for tch in range(NTCH):
                            nc.tensor.matmul(
                                ps1[j * 2 + tch][:],
                                at[:, j * 128:(j + 1) * 128],
                                xT_sb[:, hh, tch * 512:(tch + 1) * 512],
                                start=(hh == 0),
                                stop=(hh == NH - 1),
                            )
                for j in range(njw):
                    for tch in range(NTCH):
                        nc.vector.tensor_mul(
                            shrT[:, j, tch * 512:(tch + 1) * 512],
                            ps1[j * 2 + tch][:],
                            mask_sb[:, j, tch * 512:(tch + 1) * 512],
                        )

            if skip_main:
                o = op.tile([128, 512], f32, name="o_lora", tag="o")
                nc.vector.tensor_copy(o[:], shrT[:, 0, 0:512])
                nc.scalar.dma_start(out[0:128, 0:512], o[:])

            # ---- Phase 2: base GEMM + LoRA expand + bias, 8-round interleave --
            for obp in range(NOBP if not skip_main else 0):
                wq = []
                for i in range(2):
                    w = wpool.tile(
                        [128, NH, 256], bf16, name=f"wq_{obp}_{i}", tag="wq"
                    )
                    nc.scalar.dma_start(w, wp[obp * 2 + i])
                    wq.append(w)
                ps = [
                    pp.tile([128, 512], f32, name=f"mps_{obp}_{r}", tag="ps")
                    for r in range(8)
                ]
                for hh in range(NH):
                    for i in range(2):
                        for sb in range(2):
                            csl = slice(sb * 128, (sb + 1) * 128)
                            for tch in range(NTCH):
                                nc.tensor.matmul(
                                    ps[i * 4 + sb * 2 + tch][:],
                                    wq[i][:, hh, csl],
                                    xT_sb[:, hh, tch * 512:(tch + 1) * 512],
                                    start=(hh == 0),
                                    stop=(skip_lora and hh == NH - 1),
                                )
                if not skip_lora:
                    for jj in range(nbt):
                        for i in range(2):
                            for sb in range(2):
                                g = obp * 4 + i * 2 + sb
                                s = 0 if g < NGQ else (1 if g < NGQ + NGK else 2)
                                for tch in range(NTCH):
                                    nc.tensor.matmul(
                                        ps[i * 4 + sb * 2 + tch][:],
                                        bT_sb[:, jj, g * 128:(g + 1) * 128],
                                        shrT[:, s * nbt + jj,
                                             tch * 512:(tch + 1) * 512],
                                        start=False,
                                        stop=(jj == nbt - 1),
                                    )
                for i in range(2):
                    for sb in range(2):
                        g = obp * 4 + i * 2 + sb
                        for tch in range(NTCH):
                            o = op.tile(
                                [128, 512], f32,
                                name=f"o_{obp}_{i}_{sb}_{tch}", tag="o",
                            )
                            nc.vector.tensor_scalar_add(
                                o[:], ps[i * 4 + sb * 2 + tch][:],
                                bias_sb[:, g:g + 1],
                            )
                            nc.scalar.dma_start(
                                out[g * 128:(g + 1) * 128,
                                    tch * 512:(tch + 1) * 512],
                                o[:],
                            )

            if loop_ctx is not None:
                loop_ctx.__exit__(None, None, None)

            if sink is not None:
                nc.scalar.dma_start(sink[:], out[0:128, 0:512])

    nc.compile()
    return nc


def _get_nc(reps=1, timing_inputs=False, skip_lora=False, skip_main=False, lw=8):
    key = (reps, timing_inputs, skip_lora, skip_main, lw)
    if key not in _cache:
        _cache[key] = _build(
            reps=reps, timing_inputs=timing_inputs,
            skip_lora=skip_lora, skip_main=skip_main, lw=lw,
        )
    return _cache[key]


def _bf16(a):
    import ml_dtypes
    return np.ascontiguousarray(a.astype(ml_dtypes.bfloat16))


def _host_prep(x, w_qkv, b_qkv, a_q, a_k, a_v, b_q, b_k, b_v, lora_indices):
    f = np.float32
    x = np.asarray(x, f)
    li = np.asarray(lora_indices).astype(np.int64)

    perm = np.argsort(li)
    li_s = li[perm]

    lw = 8
    bases = []
    for c in range(NCORES):
        a = li_s[c * TC:(c + 1) * TC]
        span = int(a.max() - a.min() + 1)
        if span > 8:
            lw = L
        bases.append(int(min(a.min(), L - 8)))
    if lw == L:
        bases = [0] * NCORES

    wT = np.asarray(w_qkv, f).T                              # [H, OUT]
    wp = _bf16(wT.reshape(NH, 128, NOB, 256).transpose(2, 1, 0, 3))
    biasP = np.ascontiguousarray(np.asarray(b_qkv, f).reshape(NG, 128).T)

    lrw = lw * R
    nbt = lrw // 128
    njw = 3 * nbt

    a_all = [np.asarray(a, f) for a in (a_q, a_k, a_v)]      # [L, R, H]
    b_all = [np.asarray(b, f) for b in (b_q, b_k, b_v)]      # [L, out_s, R]

    in_maps = []
    for c in range(NCORES):
        base = bases[c]
        tsl = perm[c * TC:(c + 1) * TC]
        xTp = _bf16(x[tsl].T.reshape(NH, 128, TC).transpose(1, 0, 2))

        aw = np.concatenate(
            [a[base:base + lw].reshape(lrw, H) for a in a_all], axis=0
        ).T                                                  # [H, 3*lrw]
        aTp = _bf16(aw.reshape(NH, 128, njw * 128).transpose(1, 0, 2))

        bw = np.concatenate(
            [
                b[base:base + lw].transpose(0, 2, 1).reshape(lrw, -1)
                for b in b_all
            ],
            axis=1,
        )                                                    # [lrw, OUT]
        bTp = _bf16(bw.reshape(nbt, 128, OUT).transpose(1, 0, 2))

        lic = li_s[c * TC:(c + 1) * TC]
        m = (lic[None, :] == (base + np.arange(lw))[:, None])  # [lw, TC]
        m128 = np.repeat(m, R, axis=0).astype(f)               # [lrw, TC]
        mfull = np.tile(m128, (3, 1))                          # [3*lrw, TC]
        maskTp = _bf16(mfull.reshape(njw, 128, TC).transpose(1, 0, 2))

        in_maps.append(
            {
                "xTp": xTp,
                "wp": wp,
                "aTp": aTp,
                "bTp": bTp,
                "maskTp": maskTp,
                "biasP": biasP,
            }
        )
    return in_maps, perm, lw


def kernel(x, w_qkv, b_qkv, a_q, a_k, a_v, b_q, b_k, b_v, lora_indices):
    from concourse.bass_utils import run_bass_kernel_spmd

    in_maps, perm, lw = _host_prep(
        x, w_qkv, b_qkv, a_q, a_k, a_v, b_q, b_k, b_v, lora_indices
    )
    nc = _get_nc(lw=lw)
    core_ids = list(range(NCORES))
    res = run_bass_kernel_spmd(nc, in_maps, core_ids)
    out_sorted = np.concatenate(
        [res.results[c]["out"].T for c in core_ids], axis=0
    )
    out = np.empty_like(out_sorted)
    out[perm] = out_sorted
    return out
